# revision 42
# baseline (speedup 1.0000x reference)
"""Trainium2 Bass kernel for MultiHeadAttention (B=4, S=2048, D=1024, H=16, causal).

Sharding: 8 cores = data-parallel over B (4) x tensor-parallel over heads (2 groups
of 8). Core c handles batch c//2, head group c%2. Per-core dataflow (bf16 matmul
operands with fp32 PSUM accumulation, transposed layouts so no on-chip transposes):

  Qt = (wq_g @ x_q.T + bq_g)      [512, S]   (bias added on eviction, per-partition)
  Kt = (wk_g @ x_k.T + bk_g)      [512, S]
  V  = x_v @ wv_g.T               [S, 8*65]  (ones column per head; bv folded into bo_eff)
  per head pair, query chunk c (512), key-tile group (2 kt x 2 heads), causal:
     E.T quarters = Kt_h[:,kt].T @ Qt_h[:,c] -> one [128, 2048] PSUM tile
     P.T = exp(0.125 * E.T)                   (ONE ACT instr per group, PSUM->SBUF bf16)
     P.T *= mask01 on diagonal blocks         (DVE bf16)
     O_aug.T += V_aug[kt].T @ P.T            [65, 512]  (row 64 = softmax denom s)
     xh = O.T * bcast(1/s)   (bcast via K=1 f32r matmul, reciprocal_approx_fast)
  AllGather per 128-token slice (4/chunk)  -> x.T full [1024, 128] slices
  out = x.T.T @ wo_half.T + bo_eff_half      [S, 512]   (fc split by output columns)

Scheduling: proj(t+1) chains fill attention(t)'s exp windows; fc chains are
deferred to the ACT-heavy late chunks (fc(0)->chunk2, fc(1,2)->chunk3, fc(3)
pipelined against the token-sliced AllGather at the tail). DMAs are batched
(one strided descriptor per weight matrix / input chunk) because the Sync
engine issues descriptors serially at ~600ns each.

Output: host stitches column halves from the core pair of each batch.
"""

import functools
import sys

import numpy as np

sys.path.insert(0, "/opt/trn_rl_repo")

# DVE exp approx constants: exp(0.125*E) = p(E)^32, p = minimax cubic with
# p(0)=1 on |0.125*E| <= 8 (raw-energy scale folded in). Max rel err 8.9e-4.
EXP_S0 = 9.885343210669362e-09
EXP_S1 = 7.662294592591934e-06
EXP_IMM2 = 0.003906463272869587

_EXP_OPS = {}


def _register_dve_exp():
    """Register the two custom DVE ops for exp (poly pass + 5 squarings)
    through the dve_ops extension contract (OPS + name->row + spec maps)."""
    if _EXP_OPS:
        return
    import concourse.dve_ops as dve_ops
    from concourse.dve_ops import DveOp
    from concourse.dve_spec import C0, C1, C2, One, Spec, Src0, sq

    def _ref_exp_poly(in0, in1, s0, s1, imm2):
        t = in0.astype(np.float32)
        return ((t * np.float32(s0) + np.float32(s1)) * t
                + np.float32(imm2)) * t + np.float32(1.0)

    def _ref_sq5(in0, in1, s0, s1, imm2):
        v = in0.astype(np.float32)
        for _ in range(5):
            v = (v * v).astype(np.float32)
        return v

    exp_poly = DveOp(
        "EXP_POLY_ANT",
        Spec(
            body=((Src0 * C0 + C1) * Src0 + C2) * Src0 + One,
            reference=_ref_exp_poly,
        ),
        subdim=False,
        uops_sha={"v3": "64e152996f8449e5"},
    )
    sq5 = DveOp(
        "SQ5_ANT",
        Spec(body=sq(sq(sq(sq(sq(Src0))))), reference=_ref_sq5),
        subdim=False,
        uops_sha={"v3": "16919aabd4855d0a"},
    )
    for op in (exp_poly, sq5):
        if op.name not in dve_ops._SUB_OPCODE_FOR_NAME:
            dve_ops.OPS.append(op)
            dve_ops._SUB_OPCODE_FOR_NAME[op.name] = (
                dve_ops._CUSTOM_DVE_ROW_BASE + len(dve_ops.OPS) - 1)
            dve_ops.CUSTOM_DVE_SPECS[op.name] = op.spec
    _EXP_OPS["poly"] = exp_poly
    _EXP_OPS["sq5"] = sq5

# --- problem constants (hardcoded; kernel.py must be self-contained) ---
B, S, D, H, HD = 4, 2048, 1024, 16, 64
NCORES = 8
HPC = 8            # heads per core
FLOC = HPC * HD    # 512 local features per core
QCH = 512          # query chunk
KT = 128           # key tile
VW = HD + 1        # V columns per head incl. ones column (65)
NFT = FLOC // 128  # f-tiles per core (4)
NKK = D // 128     # contraction k-tiles (8)
NSL = QCH // KT    # token slices per chunk (4)
WARMUP = 20


def build_program(nc, tile, bass, mybir, seq=S):
    """Emit the per-core SPMD program into `nc` (a Bacc) under a TileContext."""
    _register_dve_exp()
    dt = mybir.dt
    f32 = dt.float32
    f32r = dt.float32r
    bf16 = dt.bfloat16
    AF = mybir.ActivationFunctionType
    ALU = mybir.AluOpType

    n_tch = seq // QCH          # token chunks
    n_ttile = seq // KT         # 128-token tiles

    # ---- I/O ----
    xqT = nc.dram_tensor("xqT", [D, seq], bf16, kind="ExternalInput").ap()
    xkT = nc.dram_tensor("xkT", [D, seq], bf16, kind="ExternalInput").ap()
    xvT = nc.dram_tensor("xvT", [D, seq], bf16, kind="ExternalInput").ap()
    wqT = nc.dram_tensor("wqT", [D, FLOC], bf16, kind="ExternalInput").ap()
    wkT = nc.dram_tensor("wkT", [D, FLOC], bf16, kind="ExternalInput").ap()
    wvT = nc.dram_tensor("wvT", [D, FLOC], bf16, kind="ExternalInput").ap()
    woT = nc.dram_tensor("woT", [D, FLOC], bf16, kind="ExternalInput").ap()
    bqc = nc.dram_tensor("bqc", [128, NFT], f32, kind="ExternalInput").ap()
    bkc = nc.dram_tensor("bkc", [128, NFT], f32, kind="ExternalInput").ap()
    bor = nc.dram_tensor("bor", [1, FLOC], f32, kind="ExternalInput").ap()
    maskin = nc.dram_tensor("maskin", [KT, KT], bf16, kind="ExternalInput").ap()
    out = nc.dram_tensor("out", [seq, FLOC], f32, kind="ExternalOutput").ap()

    with tile.TileContext(nc) as tc:
        import contextlib
        ctx = contextlib.ExitStack()
        with ctx:
            # ---------------- pools ----------------
            # PSUM budget (8 banks): eps 4 + pv0 1 + pv1 1 + mm512 2x1 = 8.
            pseps = ctx.enter_context(tc.tile_pool(name="pseps", bufs=1,
                                                   space="PSUM"))
            pspv = ctx.enter_context(tc.tile_pool(name="pspv", bufs=1,
                                                  space="PSUM"))
            psum = ctx.enter_context(tc.tile_pool(name="psum", bufs=2,
                                                  space="PSUM"))
            const = ctx.enter_context(tc.tile_pool(name="const", bufs=1))
            dram = ctx.enter_context(tc.tile_pool(name="dram", bufs=1,
                                                  space="DRAM"))
            qkv = ctx.enter_context(tc.tile_pool(name="qkv", bufs=1))
            wpool = ctx.enter_context(tc.tile_pool(name="wpool", bufs=1))
            xpool = ctx.enter_context(tc.tile_pool(name="xpool", bufs=2))
            ptpool = ctx.enter_context(tc.tile_pool(name="pt", bufs=3))
            attpool = ctx.enter_context(tc.tile_pool(name="att", bufs=4))
            xfpool = ctx.enter_context(tc.tile_pool(name="xf", bufs=3))
            ostpool = ctx.enter_context(tc.tile_pool(name="ost", bufs=3))

            # ---------------- prologue DMAs (critical order) ----------------
            # Sync issues descriptors serially (~600ns each); order by need.
            xs = {}   # (input, chunk) -> [128, NKK, QCH] tile

            def load_x(src, pfx, t):
                # per-k-tile DMAs: parallel wire across the 16 DMA engines;
                # dst [*, kk, :] is a contiguous 1KB partition-line segment.
                xt = xpool.tile([128, NKK, QCH], bf16, tag=f"x{pfx}",
                                name=f"x{pfx}{t}")
                for kk in range(NKK):
                    nc.sync.dma_start(
                        xt[:, kk, :],
                        src[kk * 128:(kk + 1) * 128, t * QCH:(t + 1) * QCH])
                xs[(pfx, t)] = xt

            def load_w(w_sb, src, eng=None):
                eng = eng or nc.sync
                for kk in range(NKK):
                    eng.dma_start(w_sb[:, kk, :],
                                  src[kk * 128:(kk + 1) * 128, :])

            # prologue loads are DMA-descriptor-issue bound (~600ns per
            # descriptor, serial per queue): split x loads (sync queue) from
            # weight loads (scalar queue, also HWDGE) so they issue in
            # parallel.
            wq_sb = wpool.tile([128, NKK, FLOC], bf16, tag="wq", name="wq")
            wk_sb = wpool.tile([128, NKK, FLOC], bf16, tag="wk", name="wk")
            wv_sb = wpool.tile([128, NKK, FLOC], bf16, tag="wv", name="wv")
            wo_sb = wpool.tile([128, NKK, FLOC], bf16, tag="wo", name="wo")
            load_w(wq_sb, wqT, eng=nc.scalar)
            load_x(xqT, "q", 0)
            bq_sb = const.tile([128, NFT], f32)
            nc.sync.dma_start(bq_sb[:], bqc[:])
            load_w(wk_sb, wkT, eng=nc.scalar)
            load_x(xkT, "k", 0)
            bk_sb = const.tile([128, NFT], f32)
            nc.sync.dma_start(bk_sb[:], bkc[:])
            load_w(wv_sb, wvT, eng=nc.scalar)
            load_x(xvT, "v", 0)
            mask_sb = const.tile([KT, KT], bf16)   # 0/1 diagonal-block mask
            nc.sync.dma_start(mask_sb[:], maskin[:])
            load_w(wo_sb, woT, eng=nc.scalar)
            bo_sb = const.tile([1, FLOC], f32r)
            nc.scalar.dma_start(bo_sb[:], bor[:].bitcast(f32r))

            # ---------------- constants (no DMA) ----------------
            ones_f = const.tile([1, QCH], f32)
            nc.vector.memset(ones_f[:], 1.0)
            ones = const.tile([1, QCH], f32r)
            nc.vector.tensor_copy(ones[:], ones_f[:])
            sel1_f = const.tile([128, HD], f32)
            nc.vector.memset(sel1_f[64:66, :], 1.0)
            nc.vector.memset(sel1_f[96:97, :], 1.0)
            sel1 = const.tile([128, HD], f32r)   # rows 64,65,96 = 1.0 (bcast lhsT)
            nc.vector.tensor_copy(sel1[64:66, :], sel1_f[64:66, :])
            nc.vector.tensor_copy(sel1[96:97, :], sel1_f[96:97, :])

            # PE warm-up: keep TensorE busy while the first DMAs land so the
            # HAM clock-gate opens before real work starts.
            warm_w = const.tile([128, 128], bf16)
            nc.vector.memset(warm_w[:], 0.0)
            warm_x = const.tile([128, QCH], bf16)
            nc.vector.memset(warm_x[:], 0.0)
            for wi in range(WARMUP):
                wp = psum.tile([128, QCH], f32, tag="mm512", name=f"warm{wi}")
                nc.tensor.matmul(wp[:], lhsT=warm_w[:], rhs=warm_x[:],
                                 start=True, stop=True)

            # HAM keepalive: when the filler deque is empty, issue junk
            # matmuls so the PE activity monitor keeps the 2.4GHz clock --
            # otherwise ACT/DVE-bound stretches re-throttle the PE to 1.2GHz
            # and all real matmuls in them run at half speed.
            _junk_n = [0]

            def junk_mm(n):
                for _ in range(n):
                    jp = psum.tile([128, QCH], f32, tag="mm512",
                                   name=f"junk{_junk_n[0]}")
                    _junk_n[0] += 1
                    nc.tensor.matmul(jp[:], lhsT=warm_w[:], rhs=warm_x[:],
                                     start=True, stop=True)

            # fc bias broadcast tile [128, FLOC] = ones.T @ bo_eff_half;
            # built lazily (before the first fc filler) so its bor DMA is
            # never on the prologue critical path.
            bo_bc = const.tile([128, FLOC], f32)

            def build_bo_bc():
                bp = psum.tile([128, QCH], f32, tag="mm512", name="bobc")
                nc.tensor.matmul(bp[:], lhsT=ones[0:1, 0:128],
                                 rhs=bo_sb[0:1, :], start=True, stop=True)
                nc.vector.tensor_copy(bo_bc[:], bp[:])

            # persistent projection outputs
            qt_tiles = [qkv.tile([128, seq], bf16, tag=f"qt{i}", name=f"qt{i}")
                        for i in range(NFT)]
            kt_tiles = [qkv.tile([128, seq], bf16, tag=f"kt{i}", name=f"kt{i}")
                        for i in range(NFT)]
            v_tiles = [qkv.tile([KT, HPC * VW], bf16, tag=f"v{i}", name=f"v{i}")
                       for i in range(n_ttile)]

            # DRAM bounce buffers for the per-chunk AllGather (bf16)
            ag_in = [dram.tile([FLOC, QCH], bf16, tag=f"agi{c}",
                               name=f"agi{c}") for c in range(n_tch)]
            ag_out = [dram.tile([2 * FLOC, QCH], bf16, tag=f"ago{c}",
                                name=f"ago{c}") for c in range(n_tch)]

            def proj_qk_chain(pfx, w_sb, bias_sb, dst, t, f):
                xts = xs[(pfx, t)]
                pp = psum.tile([128, QCH], f32, tag="mm512", name=f"pp{pfx}{t}{f}")
                for kk in range(NKK):
                    nc.tensor.matmul(
                        pp[:], lhsT=w_sb[:, kk, f * 128:(f + 1) * 128],
                        rhs=xts[:, kk, :],
                        start=(kk == 0), stop=(kk == NKK - 1))
                nc.scalar.activation(
                    dst[f][:, t * QCH:(t + 1) * QCH], pp[:],
                    AF.Identity, bias=bias_sb[:, f:f + 1])

            def proj_v_chain(t, tt):
                g = t * NSL + tt
                xts = xs[("v", t)]
                pp = psum.tile([128, FLOC], f32, tag="mm512", name=f"ppv{g}")
                for kk in range(NKK):
                    nc.tensor.matmul(
                        pp[:], lhsT=xts[:, kk, tt * KT:(tt + 1) * KT],
                        rhs=wv_sb[:, kk, :],
                        start=(kk == 0), stop=(kk == NKK - 1))
                vv = v_tiles[g].rearrange("p (h e) -> p h e", e=VW)
                nc.scalar.copy(
                    vv[:, :, 0:HD], pp[:].rearrange("p (h d) -> p h d", d=HD))
                nc.gpsimd.memset(vv[:, :, HD:VW], 1.0)

            def proj_chunk_fillers(t):
                fillers = []
                for f in range(NFT):
                    fillers.append(
                        lambda f=f: proj_qk_chain("q", wq_sb, bq_sb, qt_tiles,
                                                  t, f))
                    fillers.append(
                        lambda f=f: proj_qk_chain("k", wk_sb, bk_sb, kt_tiles,
                                                  t, f))
                for tt in range(NSL):
                    fillers.append(lambda tt=tt: proj_v_chain(t, tt))
                return fillers

            def attention_pair(c, hp, fill=None):
                ft = hp
                heads = (2 * hp, 2 * hp + 1)
                pvs = {}
                for h in heads:
                    pvs[h] = pspv.tile([VW, QCH], f32, tag=f"pv{h % 2}",
                                       name=f"pv{c}_{h}", bufs=1)
                nkt = NSL * (c + 1)     # causal key tiles
                for grp in range(nkt // 2):
                    # per-head eps (2 banks each) so the two heads' E->exp
                    # chains pipeline independently; h0's exp runs on ACT,
                    # h1's on DVE (custom poly+squarings op pair).
                    eps = [pseps.tile([128, 2 * QCH], f32, tag=f"eps{hl}",
                                      name=f"ep{c}_{hp}_{grp}_{hl}", bufs=1)
                           for hl in range(2)]
                    for j2 in range(2):
                        kt = grp * 2 + j2
                        band = kt - NSL * c
                        off = band * KT if band > 0 else 0   # causal trim
                        for hloc in range(2):
                            fr = hloc * HD
                            qoff = j2 * QCH
                            nc.tensor.matmul(
                                eps[hloc][:, qoff + off:qoff + QCH],
                                lhsT=kt_tiles[ft][fr:fr + HD,
                                                  kt * KT:(kt + 1) * KT],
                                rhs=qt_tiles[ft][fr:fr + HD,
                                                 c * QCH + off:(c + 1) * QCH],
                                start=True, stop=True)
                    pts = []
                    for hl in range(2):
                        pt = ptpool.tile([128, 2 * QCH], bf16, tag=f"pt{hl}",
                                         name=f"pt{c}_{hp}_{grp}_{hl}")
                        pts.append(pt)
                    # exp split by measured engine rates: ACT takes h0 plus
                    # h1's first key tile; the custom-DVE pair takes h1's
                    # second key tile (aligned to the j2 halves so each PV
                    # waits on exactly one producer).
                    SPL = QCH
                    nc.scalar.activation(pts[0][:], eps[0][:], AF.Exp,
                                         scale=0.125)
                    nc.scalar.activation(pts[1][:, 0:SPL], eps[1][:, 0:SPL],
                                         AF.Exp, scale=0.125)
                    scr = attpool.tile([128, 2 * QCH - SPL], f32, tag="escr",
                                       name=f"escr{c}_{hp}_{grp}", bufs=1)
                    nc.vector._custom_dve(_EXP_OPS["poly"], out=scr[:],
                                          in0=eps[1][:, SPL:2 * QCH],
                                          s0=EXP_S0, s1=EXP_S1, imm2=EXP_IMM2)
                    nc.vector._custom_dve(_EXP_OPS["sq5"],
                                          out=pts[1][:, SPL:2 * QCH],
                                          in0=scr[:])
                    for j2 in range(2):
                        kt = grp * 2 + j2
                        band = kt - NSL * c
                        if band >= 0:   # mask the diagonal block (on GpSimd)
                            for hloc in range(2):
                                qoff = j2 * QCH
                                sl = pts[hloc][:, qoff + band * KT:
                                               qoff + (band + 1) * KT]
                                nc.gpsimd.tensor_tensor(sl, sl, mask_sb[:],
                                                        ALU.mult)
                    for j2 in range(2):
                        kt = grp * 2 + j2
                        band = kt - NSL * c
                        off = band * KT if band > 0 else 0
                        for hloc in range(2):
                            h = heads[hloc]
                            qoff = j2 * QCH
                            nc.tensor.matmul(
                                pvs[h][:, off:QCH],
                                lhsT=v_tiles[kt][:, :].rearrange(
                                    "p (h e) -> p h e", e=VW)[:, h, :],
                                rhs=pts[hloc][:, qoff + off:qoff + QCH],
                                start=(kt == 0), stop=(kt == nkt - 1),
                                skip_group_check=True)
                    if fill is not None:
                        fill()   # slot one indep. GEMM chain into the window
                # normalize: xh = O.T * bcast(1/s)
                for h in heads:
                    pv = pvs[h]
                    sr = attpool.tile([128, QCH], f32r, tag="sr", name=f"sr{c}_{h}")
                    nc.vector.tensor_copy(sr[64:65, :], pv[HD:VW, :])
                    bc = psum.tile([HD, QCH], f32, tag="mm512", name=f"bc{c}_{h}")
                    nc.tensor.matmul(bc[:], lhsT=sel1[64:65, :], rhs=sr[64:65, :],
                                     start=True, stop=True)
                    rcp = attpool.tile([HD, QCH], f32, tag="rcp", name=f"rcp{c}_{h}")
                    nc.vector.reciprocal_approx_fast(rcp[:], bc[:])
                    xh = attpool.tile([HD, QCH], bf16, tag="xh", name=f"xh{c}_{h}")
                    nc.vector.tensor_tensor(xh[:], pv[0:HD, :], rcp[:], ALU.mult)
                    nc.sync.dma_start(ag_in[c][h * HD:(h + 1) * HD, :], xh[:])

            xf_tiles = {}

            def fc_prefetch(c):
                """Emit the xf DMAs for fc of chunk c (gated on AG(c))."""
                xt = xfpool.tile([128, NKK, QCH], bf16, tag="xf",
                                 name=f"xf{c}")
                for kk in range(NKK):
                    nc.sync.dma_start(xt[:, kk, :],
                                      ag_out[c][kk * 128:(kk + 1) * 128, :])
                xf_tiles[c] = xt

            def fc_chunk_fillers(c):
                """Fillers for fc of chunk c: one chain per 128-token slice."""

                def chain(tt):
                    xt = xf_tiles[c]
                    fp = psum.tile([128, QCH], f32, tag="mm512", name=f"fp{c}_{tt}")
                    for kk in range(NKK):
                        nc.tensor.matmul(
                            fp[:], lhsT=xt[:, kk, tt * KT:(tt + 1) * KT],
                            rhs=wo_sb[:, kk, :],
                            start=(kk == 0), stop=(kk == NKK - 1))
                    ost = ostpool.tile([128, QCH], f32, tag="ost",
                                       name=f"ost{c}_{tt}")
                    nc.vector.tensor_tensor(ost[:], fp[:], bo_bc[:], ALU.add)
                    nc.sync.dma_start(
                        out[c * QCH + tt * KT:c * QCH + (tt + 1) * KT, :], ost[:])

                return [lambda tt=tt: chain(tt) for tt in range(NSL)]

            def ag_emit(t):
                nc.gpsimd.collective_compute(
                    "AllGather", ALU.bypass,
                    replica_groups=[[0, 1], [2, 3], [4, 5], [6, 7]],
                    ins=[ag_in[t].opt()], outs=[ag_out[t].opt()])

            # ---------------- software-pipelined chunk loop ----------------
            import collections as _cl
            p0 = proj_chunk_fillers(0)
            # emit q0,k0 chains + all V chains before attention(0); the f1-3
            # q/k chains of chunk 0 become fillers inside attention(0)
            # (pair hp only needs f-tile hp, produced 2+ groups earlier).
            for fl in [p0[0], p0[1]] + p0[8:12]:
                fl()
            carry = _cl.deque(p0[2:8])
            for t in range(n_tch):
                fillers = carry
                carry = _cl.deque()
                if t + 1 < n_tch:
                    load_x(xqT, "q", t + 1)
                    load_x(xkT, "k", t + 1)
                    load_x(xvT, "v", t + 1)
                    fillers.extend(proj_chunk_fillers(t + 1))
                if t == 2:
                    build_bo_bc()
                    fc_prefetch(0)
                    fillers.extend(fc_chunk_fillers(0))
                if t == 3:
                    fc_prefetch(1)
                    fc_prefetch(2)
                    fillers.extend(fc_chunk_fillers(1))
                    fillers.extend(fc_chunk_fillers(2))

                def fill(fillers=fillers):
                    if fillers:
                        fillers.popleft()()
                    else:
                        junk_mm(5)

                for hp in range(HPC // 2):
                    attention_pair(t, hp, fill=fill)
                while fillers:
                    fillers.popleft()()
                ag_emit(t)
            fc_prefetch(n_tch - 1)
            junk_mm(48)   # keep the PE warm across the last AllGather wait
            for fl in fc_chunk_fillers(n_tch - 1):
                fl()   # epilogue: last chunk's fc, pipelined vs its AllGather
    return nc


@functools.lru_cache(maxsize=None)
def _compiled(seq=S):
    import concourse.bacc as bacc
    import concourse.bass as bass
    import concourse.mybir as mybir
    import concourse.tile as tile

    nc = bacc.Bacc("TRN2", target_bir_lowering=False, debug=False,
                   num_devices=NCORES)
    build_program(nc, tile, bass, mybir, seq=seq)
    nc.compile()
    return nc


def _host_prep(inputs, seq=S):
    """Build the 8 per-core input maps from full inputs."""
    import ml_dtypes
    bf16 = ml_dtypes.bfloat16

    q, k, v = inputs["query"], inputs["key"], inputs["value"]
    wq, bq = inputs["wq"], inputs["bq"]
    wk, bk = inputs["wk"], inputs["bk"]
    wv, bv = inputs["wv"], inputs["bv"]
    wo, bo = inputs["wo"], inputs["bo"]

    f32 = np.float32
    bo_eff = (bo + wo @ bv).astype(f32)

    # 0/1 diagonal-block mask [128, 128]
    kk = np.arange(KT)[:, None]
    qq = np.arange(KT)[None, :]
    mask = (qq >= kk).astype(bf16)

    in_maps = []
    for core in range(NCORES):
        b, g = core // 2, core % 2
        sl = slice(g * FLOC, (g + 1) * FLOC)
        in_maps.append({
            "xqT": np.ascontiguousarray(q[b, :seq].T).astype(bf16),
            "xkT": np.ascontiguousarray(k[b, :seq].T).astype(bf16),
            "xvT": np.ascontiguousarray(v[b, :seq].T).astype(bf16),
            "wqT": np.ascontiguousarray(wq[sl].T).astype(bf16),
            "wkT": np.ascontiguousarray(wk[sl].T).astype(bf16),
            "wvT": np.ascontiguousarray(wv[sl].T).astype(bf16),
            "woT": np.ascontiguousarray(wo[sl].T).astype(bf16),
            "bqc": np.ascontiguousarray(bq[sl].reshape(NFT, 128).T).astype(f32),
            "bkc": np.ascontiguousarray(bk[sl].reshape(NFT, 128).T).astype(f32),
            "bor": bo_eff[sl].reshape(1, FLOC),
            "maskin": mask,
        })
    return in_maps


def run(inputs, seq=S, trace=False):
    from concourse.bass_utils import run_bass_kernel_spmd

    nc = _compiled(seq)
    in_maps = _host_prep(inputs, seq)
    res = run_bass_kernel_spmd(nc, in_maps, core_ids=list(range(NCORES)),
                               trace=trace)
    out = np.zeros((B, seq, D), np.float32)
    for b in range(B):
        out[b, :, 0:FLOC] = res.results[2 * b]["out"]
        out[b, :, FLOC:D] = res.results[2 * b + 1]["out"]
    return out, res


def kernel(**inputs):
    inputs = {k: np.asarray(v) for k, v in inputs.items()}
    out, _ = run(inputs)
    return out
